# revision 59
# baseline (speedup 1.0000x reference)
"""Self-contained Trainium2 Bass kernel for nn_Attention (dense transformer MHA block).

Full inputs in, full outputs out. Sharding: batch (B=8) data-parallel across the
8 NeuronCores -- one batch element per core. Weights are NOT replicated from the
host: each core receives a 1/8 row-shard of the combined [qkv_w ; proj_w] matrix
(in bf16) and the full weights are reconstructed on-chip with 5 pipelined
AllGathers over NeuronLink, ordered by first use (q/k pairs 0-2, v, q/k pairs
3-5, proj). This cuts host->device traffic from 100.7 MB to ~17.3 MB per call;
the output returns as bf16 (12.6 MB instead of 25.2 MB).

Per-core math (x: [1024, 768], H=12 heads, D=64):
  qkv = x @ qkv_w.T ; q,k,v split ; per head: softmax(q k^T / 8) v ; proj + bias.

Layout/precision strategy:
  - All matmul operands in bfloat16 (2x PE throughput vs float32r); PSUM
    accumulation stays fp32. End-to-end relative error ~6e-3 vs the fp32
    reference (harness gate 2e-2).
  - x^T and W^T produced on-chip via PE transposes.
  - q^T,k^T computed in [o, i] layout -> directly usable as the
    S^T = k^T.T @ q^T matmul operands (contraction over d on partitions).
  - v computed in natural [token, feature] layout with an extra ones column;
    O' = [v | 1].T @ E^T yields the attention output AND the softmax row-sums
    in one matmul (65-column trick). v columns are produced per head pair so
    pair p's O' only waits for v-weight block p.
  - softmax without max-subtraction (scores ~N(0,1); fp32 exp is safe).
  - normalization without a DRAM round-trip (DMA triggers hold their issuing
    queue in this pipeline): DVE copies the PSUM rowsum row, the otherwise
    idle GPSIMD/Pool engine partition_broadcasts it, DVE takes an approx
    reciprocal in place and multiplies straight out of PSUM into attnT.
  - schedule: weight-shard bounce + 5 AllGathers issue first. All weight
    loads are prefetched on the sync DMA queue in gather-completion order so
    no load head-of-line blocks a ready one. The x-transpose prelude overlaps
    gather 0; scores run ahead of the O' pass (10 E^T tiles) so the Act
    engine rarely starves; proj-weight transposes and the proj passes run
    after the attention loop (they depend on the last gather and must not
    block the PE queue); proj sums cb 0-3 early and cb 4-5 late.
  - attnout lands directly in [feature, token] layout = proj's lhsT; proj
    output is natural [token, feature] and DMAs straight out.

Host-side weight slab layout (w_cat, 3072 rows of 768; 24 128-row blocks;
qi = qkv_w rows i*128..., ki = qkv_w rows 768+i*128..., vj = qkv_w rows
1536+j*128..., pj = proj_w rows j*128...):
  slab 0 (blocks  0- 3): q0,k0,q1,k1
  slab 1 (blocks  4- 7): q2,k2,v0,v1
  slab 2 (blocks  8-11): v2,v3,v4,v5
  slab 3 (blocks 12-17): q3,k3,q4,k4,q5,k5
  slab 4 (blocks 18-23): p0..p5
Core c's shard = the c-th 1/8 row-chunk of each slab, stacked -> [384, 768].
AllGather k (rank-ordered concat of the 8 cores' chunk k) reproduces slab k.
"""

import os
import sys

for _p in ("/opt/trn_rl_repo",):
    if os.path.isdir(_p) and _p not in sys.path:
        sys.path.insert(0, _p)

import numpy as np

P = 128
N = 1024          # tokens per batch element
C = 768           # model dim
H = 12            # heads
D = 64            # head dim
B = 8             # batch (== n cores)
NB = N // P       # 8 token blocks
CB = C // P       # 6 feature blocks
SCALE = D ** -0.5  # 0.125
# uneven gather slabs (in 128-row blocks): collectives cost ~15us fixed +
# bytes/40GB/s, so slab boundaries are placed where the schedule needs the
# data: [q0,k0,q1,k1 | q2,k2,v0,v1 | v2..v5 | q3,k3,q4,k4,q5,k5 | proj]
SLAB_BLOCKS = [4, 4, 4, 6, 6]
NSLAB = len(SLAB_BLOCKS)
SLAB_START = [sum(SLAB_BLOCKS[:k]) for k in range(NSLAB)]   # block offsets
CHUNKS = [b * P // B for b in SLAB_BLOCKS]                  # per-core rows
CHUNK_START = [sum(CHUNKS[:k]) for k in range(NSLAB)]
SHARD_ROWS = sum(CHUNKS)                                    # 384


def build_attention_bass():
    import concourse.mybir as mybir
    import concourse.tile as tile
    from concourse import bacc
    from concourse.masks import make_identity

    f32 = mybir.dt.float32
    bf16 = mybir.dt.bfloat16
    nc = bacc.Bacc("TRN2", target_bir_lowering=False, debug=False)

    x = nc.dram_tensor("x", [N, C], bf16, kind="ExternalInput")
    w_shard = nc.dram_tensor(
        "w_shard", [SHARD_ROWS, C], bf16, kind="ExternalInput")
    proj_b = nc.dram_tensor("proj_b", [C], f32, kind="ExternalInput")
    out = nc.dram_tensor("out", [N, C], bf16, kind="ExternalOutput")

    # collective bounce buffers (collectives can't touch I/O tensors)
    w_bounce = [
        nc.dram_tensor(f"w_bounce{k}", [CHUNKS[k], C], bf16)
        for k in range(NSLAB)]
    w_slab = [
        nc.dram_tensor(f"w_slab{k}", [SLAB_BLOCKS[k] * P, C], bf16,
                       addr_space="Shared")
        for k in range(NSLAB)
    ]

    x_r = x.rearrange("(nb p) c -> nb p c", p=P)        # [8, 128, 768]
    slab_r = [w_slab[k].rearrange("(ob p) c -> ob p c", p=P)
              for k in range(NSLAB)]
    out_r = out.rearrange("(nb p) c -> nb p c", p=P)

    # global block order: [q0,k0,q1,k1 | q2,k2,v0,v1 | v2..v5 |
    #                      q3,k3,q4,k4,q5,k5 | p0..p5] -- 24 blocks
    def blk(g):
        for k in range(NSLAB - 1, -1, -1):
            if g >= SLAB_START[k]:
                return slab_r[k][g - SLAB_START[k]]
        raise AssertionError(g)

    def qk_src(hb, qk):
        """row-block source for pair hb's q (qk=0) / k (qk=1) weights"""
        m = 2 * hb + qk
        return blk(m if m < 6 else m + 6)  # qk 6..11 live after the 6 v blocks

    def v_src(j):
        return blk(6 + j)

    def p_src(j):
        return blk(18 + j)

    with tile.TileContext(nc) as tc:
        with tc.tile_pool(name="persist", bufs=1) as pA:
            # ---- long-lived tensors
            vext = pA.tile([P, NB, H, D + 1], bf16)    # v natural + ones col
            ident_f = pA.tile([P, P], f32)
            ident = pA.tile([P, P], bf16)
            attnT = [pA.tile([P, N], bf16, name=f"attnT{i}") for i in range(CB)]
            pwT = pA.tile([P, CB, C], bf16)            # proj_w^T [c, cb, o2]
            wTv = pA.tile([P, CB, C], bf16)            # v-part of qkv_w^T
            # x^T in two i-halves for finer-grained dependencies
            xTh = [pA.tile([P, CB, 512], bf16, name=f"xTh{i}") for i in range(2)]
            bias_bc = pA.tile([P, C], f32)             # proj_b broadcast

            ones_f = pA.tile([P, NB * H], bf16)
            make_identity(nc, ident_f[:])
            nc.vector.tensor_copy(ident[:], ident_f[:])
            nc.vector.memset(ones_f[:], 1.0)
            nc.vector.tensor_copy(
                vext[:, :, :, D:D + 1],
                ones_f[:].rearrange("p (nb h) -> p nb h", nb=NB)[:, :, :, None])
            # bias broadcast BEFORE the collective triggers: a trigger holds
            # the gpsimd queue until the collective engine picks it up.
            nc.gpsimd.dma_start(bias_bc[:], proj_b[None, :].to_broadcast((P, C)))

            # ---- weight shard bounce + pipelined AllGathers
            with nc.named_scope("allgather"):
                for k in range(NSLAB):
                    nc.sync.dma_start(
                        w_bounce[k][:, :],
                        w_shard[CHUNK_START[k]:CHUNK_START[k] + CHUNKS[k], :])
                for k in range(NSLAB):
                    nc.gpsimd.collective_compute(
                        "AllGather", mybir.AluOpType.bypass,
                        replica_groups=[list(range(B))],
                        ins=[w_bounce[k].ap().opt()],
                        outs=[w_slab[k].ap().opt()],
                    )

            # ============ prelude: x -> x^T (48 PE transposes) ============
            with tc.tile_pool(name="pre_roll", bufs=4) as p_roll, \
                 tc.tile_pool(name="tpsx", bufs=6, space="PSUM") as tpsx, \
                 nc.named_scope("x_transpose"):
                for nbg in range(2):
                    xnat = []
                    for j in range(4):
                        t = p_roll.tile([P, C], bf16, tag="xnat")
                        nc.sync.dma_start(t[:], x_r[nbg * 4 + j])
                        xnat.append(t)
                    for cb in range(CB):
                        pst = tpsx.tile([P, 512], bf16, tag="tpsx")
                        for j in range(4):
                            nc.tensor.transpose(
                                pst[:, j * P:(j + 1) * P],
                                xnat[j][:, cb * P:(cb + 1) * P],
                                ident[:])
                        nc.any.tensor_copy(xTh[nbg][:, cb, :], pst[:])

            # ============ attention: pipelined pairs ============
            with tc.tile_pool(name="wload", bufs=1) as p_wl, \
                 tc.tile_pool(name="wtq", bufs=3) as p_wtq, \
                 tc.tile_pool(name="qkroll", bufs=3) as p_qk, \
                 tc.tile_pool(name="etpool", bufs=10) as p_et, \
                 tc.tile_pool(name="ph2sm", bufs=1) as p_sm, \
                 tc.tile_pool(name="mm1", bufs=2, space="PSUM") as mm1p, \
                 tc.tile_pool(name="pss", bufs=2, space="PSUM") as pssp, \
                 tc.tile_pool(name="pso", bufs=1, space="PSUM") as psop, \
                 nc.named_scope("attention"):

                # ---- prefetch ALL weight row-blocks on the sync queue in
                # gather-completion order: qk pairs 0-2, v, qk pairs 3-5, proj.
                wnat = {}
                vnat = {}
                pnat = {}

                def load_qk(hb, qk):
                    t = p_wl.tile([P, C], bf16, name=f"wq{hb}_{qk}")
                    nc.sync.dma_start(t[:], qk_src(hb, qk))
                    wnat[(hb, qk)] = t

                def load_v(j):
                    t = p_wl.tile([P, C], bf16, name=f"wv{j}")
                    nc.sync.dma_start(t[:], v_src(j))
                    vnat[j] = t

                def load_p(j):
                    t = p_wl.tile([P, C], bf16, name=f"wp{j}")
                    nc.sync.dma_start(t[:], p_src(j))
                    pnat[j] = t

                for hb in range(3):          # slabs 0-1
                    load_qk(hb, 0)
                    load_qk(hb, 1)
                for j in range(CB):          # slabs 1-2
                    load_v(j)
                for hb in range(3, 6):       # slabs 3-4
                    load_qk(hb, 0)
                    load_qk(hb, 1)
                for j in range(CB):          # slabs 4-5
                    load_p(j)

                def transpose_w_block(dst3, src, obi):
                    """PE-transpose one [128, 768] natural row-block into
                    dst3[:, :, obi*128:(obi+1)*128] ([c, cb, o] layout)."""
                    psa = mm1p.tile([P, 512], bf16, tag="mm1")
                    for cb in range(4):
                        nc.tensor.transpose(
                            psa[:, cb * P:(cb + 1) * P],
                            src[:, cb * P:(cb + 1) * P], ident[:])
                    nc.vector.tensor_copy(
                        dst3[:, 0:4, obi * P:(obi + 1) * P],
                        psa[:].rearrange("p (cb k) -> p cb k", cb=4))
                    psb = mm1p.tile([P, 512], bf16, tag="mm1")
                    for cb in range(2):
                        nc.tensor.transpose(
                            psb[:, cb * P:(cb + 1) * P],
                            src[:, (4 + cb) * P:(5 + cb) * P], ident[:])
                    nc.vector.tensor_copy(
                        dst3[:, 4:6, obi * P:(obi + 1) * P],
                        psb[:, 0:256].rearrange("p (cb k) -> p cb k", cb=2))

                def qk_transpose(hb, qk):
                    wtq = p_wtq.tile([P, CB, P], bf16, tag="wqk")
                    transpose_w_block(wtq, wnat.pop((hb, qk))[:], 0)
                    return wtq

                def qk_matmuls(wtq):
                    """q^T or k^T [o=128, i=1024] from transposed weights."""
                    t = p_qk.tile([P, N], bf16, tag="qkt")
                    for ic in range(2):
                        ps1 = mm1p.tile([P, 512], f32, tag="mm1")
                        for cb in range(CB):
                            nc.tensor.matmul(
                                ps1[:], wtq[:, cb, 0:P],
                                xTh[ic][:, cb, :],
                                start=(cb == 0), stop=(cb == CB - 1))
                        nc.vector.tensor_copy(
                            t[:, ic * 512:(ic + 1) * 512], ps1[:])
                    return t

                def head_scores(qt, kt, hp):
                    """S^T = k^T.T @ q^T ; E^T = exp(S^T/8) for one head."""
                    r0, r1 = hp * D, hp * D + D
                    ets = []
                    for jbg in range(4):
                        et = p_et.tile([P, 2, N], bf16, tag="et")
                        ets.append(et)
                        for jj in range(2):
                            jb = jbg * 2 + jj
                            ps_s = pssp.tile([P, N], f32, tag="pss")
                            for ic in range(2):
                                nc.tensor.matmul(
                                    ps_s[:, ic * 512:(ic + 1) * 512],
                                    kt[r0:r1, jb * P:(jb + 1) * P],
                                    qt[r0:r1, ic * 512:(ic + 1) * 512],
                                    start=True, stop=True)
                            nc.scalar.activation(
                                et[:, jj, :], ps_s[:],
                                mybir.ActivationFunctionType.Exp, scale=SCALE)
                    return ets

                def v_block(j):
                    """vext columns for heads 2j, 2j+1 (v-weight block j)."""
                    transpose_w_block(wTv, vnat.pop(j)[:], j)
                    for jb in range(NB):
                        ps2 = mm1p.tile([P, 512], f32, tag="mm1")
                        for cb in range(CB):
                            nc.tensor.matmul(
                                ps2[:, 0:P],
                                xTh[jb // 4][:, cb,
                                             (jb % 4) * P:(jb % 4 + 1) * P],
                                wTv[:, cb, j * P:(j + 1) * P],
                                start=(cb == 0), stop=(cb == CB - 1))
                        nc.any.tensor_copy(
                            vext[:, jb, 2 * j:2 * j + 2, 0:D],
                            ps2[:, 0:P].rearrange("p (h d) -> p h d", d=D))

                def head_out(ets, h, hb, hp):
                    """O'^T = [v|1].T @ E^T ; normalize out of PSUM: rowsum
                    copy + partition_broadcast on GPSIMD (idle engine),
                    reciprocal+multiply on DVE. No DRAM round-trip (DMA
                    triggers hold their issuing queue in this pipeline)."""
                    r0, r1 = hp * D, hp * D + D
                    ps_o = psop.tile([D + 1, N], f32, tag="pso")
                    for jb in range(NB):
                        for ic in range(2):
                            nc.tensor.matmul(
                                ps_o[:, ic * 512:(ic + 1) * 512],
                                vext[:, jb, h, :],
                                ets[jb // 2][:, jb % 2, ic * 512:(ic + 1) * 512],
                                start=(jb == 0), stop=(jb == NB - 1))
                    rsum = p_sm.tile([1, N], f32, tag="rsum", bufs=1)
                    rb = p_sm.tile([P, N], f32, tag="rb", bufs=1)
                    nc.vector.tensor_copy(rsum[:], ps_o[D:D + 1, :])
                    nc.gpsimd.partition_broadcast(rb[:], rsum[:])
                    nc.vector.reciprocal_approx_fast(out=rb[:], in_=rb[:])
                    nc.vector.tensor_tensor(
                        attnT[hb][r0:r1, :], ps_o[0:D, :],
                        rb[r0:r1, :], mybir.AluOpType.mult)

                # ---- pipelined schedule, ordered so nothing that is ready
                # queues behind a gather wait (queues are in-order): pairs
                # 0-2 and ALL v blocks are consumed before pair 3's weight
                # transposes (which wait on the last qkv gather).
                def pair_scores(hb):
                    # both transposes first: k's PE transposes overlap q's
                    # PSUM-evacuation copies on DVE
                    wtq_q = qk_transpose(hb, 0)
                    wtq_k = qk_transpose(hb, 1)
                    qt = qk_matmuls(wtq_q)
                    kt = qk_matmuls(wtq_k)
                    return (head_scores(qt, kt, 0), head_scores(qt, kt, 1))

                def pair_out(hb, ets2):
                    head_out(ets2[0], 2 * hb, hb, 0)
                    head_out(ets2[1], 2 * hb + 1, hb, 1)

                ets0 = pair_scores(0)          # weights @ gather 0
                ets1 = pair_scores(1)          # weights @ gather 0
                v_block(0)                     # v0 @ gather 1
                pair_out(0, ets0)
                v_block(1)                     # v1 @ gather 1
                pair_out(1, ets1)
                ets2 = pair_scores(2)          # weights @ gather 1
                v_block(2)                     # v2 @ gather 2
                pair_out(2, ets2)
                for j in range(3, CB):
                    v_block(j)                 # v3-5 @ gather 2: fill the
                                               # PE queue during gather 3
                for hb in range(3, CB):        # weights @ gather 3
                    pair_out(hb, pair_scores(hb))

                # ---- proj, two passes: cb 0..4 as soon as pairs 0-4 are
                # normalized; the cb=5 contribution lands after the final
                # pair normalizes.
                with nc.named_scope("proj"):
                    for j in range(CB):
                        transpose_w_block(pwT, pnat.pop(j)[:], j)
                    osbs = []
                    for nb in range(NB):
                        osb = p_sm.tile([P, C], f32, tag="osb", bufs=8)
                        osbs.append(osb)
                        for (o0, w) in ((0, 512), (512, 256)):
                            ps3 = mm1p.tile([P, 512], f32, tag="mm1")
                            for cb in range(CB - 2):
                                nc.tensor.matmul(
                                    ps3[:, 0:w],
                                    attnT[cb][:, nb * P:(nb + 1) * P],
                                    pwT[:, cb, o0:o0 + w],
                                    start=(cb == 0), stop=(cb == CB - 3))
                            nc.vector.tensor_tensor(
                                osb[:, o0:o0 + w], ps3[:, 0:w],
                                bias_bc[:, o0:o0 + w], mybir.AluOpType.add)
                    for nb in range(NB):
                        osb = osbs[nb]
                        osb16 = p_sm.tile([P, C], bf16, tag="osb16", bufs=2)
                        for (o0, w) in ((0, 512), (512, 256)):
                            ps3 = mm1p.tile([P, 512], f32, tag="mm1")
                            for cb in (CB - 2, CB - 1):
                                nc.tensor.matmul(
                                    ps3[:, 0:w],
                                    attnT[cb][:, nb * P:(nb + 1) * P],
                                    pwT[:, cb, o0:o0 + w],
                                    start=(cb == CB - 2), stop=(cb == CB - 1))
                            nc.vector.tensor_tensor(
                                osb16[:, o0:o0 + w],
                                osb[:, o0:o0 + w],
                                ps3[:, 0:w], mybir.AluOpType.add)
                        nc.sync.dma_start(out_r[nb], osb16[:])

    nc.finalize()
    return nc


_NC_CACHE = None
_FAST_CACHE = None


def _build_fast_runner(nc):
    """Axon-path runner: like bass2jax.run_bass_via_pjrt, but the donated
    output buffers are created ON DEVICE (a host np.zeros would be shipped
    over the tunnel every call -- 12.6 MB here) and the shard_map'd jit is
    built once and cached (run_bass_via_pjrt re-traces every call)."""
    import jax
    import jax.numpy as jnp
    from jax.sharding import Mesh, PartitionSpec, NamedSharding
    from jax.experimental.shard_map import shard_map
    import concourse.mybir as mybir
    from concourse.bass2jax import (
        _bass_exec_p, install_neuronx_cc_hook, partition_id_tensor)

    install_neuronx_cc_hook()
    devices = jax.devices()[:B]
    assert len(devices) == B
    mesh = Mesh(np.asarray(devices), ("core",))

    partition_name = (
        nc.partition_id_tensor.name if nc.partition_id_tensor else None)
    in_names, out_names, out_avals = [], [], []
    for alloc in nc.m.functions[0].allocations:
        if not isinstance(alloc, mybir.MemoryLocationSet):
            continue
        name = alloc.memorylocations[0].name
        if alloc.kind == "ExternalInput":
            if name != partition_name:
                in_names.append(name)
        elif alloc.kind == "ExternalOutput":
            out_names.append(name)
            out_avals.append(jax.core.ShapedArray(
                tuple(alloc.tensor_shape), mybir.dt.np(alloc.dtype)))
    n_params = len(in_names)
    n_outs = len(out_avals)
    all_in_names = list(in_names) + list(out_names)
    if partition_name is not None:
        all_in_names.append(partition_name)

    def _body(*args):
        operands = list(args)
        if partition_name is not None:
            operands.append(partition_id_tensor())
        return tuple(_bass_exec_p.bind(
            *operands, out_avals=tuple(out_avals),
            in_names=tuple(all_in_names), out_names=tuple(out_names),
            lowering_input_output_aliases=(),
            sim_require_finite=True, sim_require_nnan=True, nc=nc))

    in_specs = (PartitionSpec("core"),) * (n_params + n_outs)
    out_specs = (PartitionSpec("core"),) * n_outs
    sharded = jax.jit(
        shard_map(_body, mesh=mesh, in_specs=in_specs, out_specs=out_specs,
                  check_rep=False),
        donate_argnums=tuple(range(n_params, n_params + n_outs)),
        keep_unused=True)

    zero_shardings = tuple(
        NamedSharding(mesh, PartitionSpec("core")) for _ in out_avals)
    make_zeros = jax.jit(
        lambda: tuple(jnp.zeros((B * av.shape[0], *av.shape[1:]), av.dtype)
                      for av in out_avals),
        out_shardings=zero_shardings)

    in_sharding = NamedSharding(mesh, PartitionSpec("core"))

    def run(concat_in):
        outs = sharded(*concat_in, *make_zeros())
        return [np.asarray(o).reshape(B, *av.shape)
                for o, av in zip(outs, out_avals)]

    return in_names, in_sharding, run


def _shard_inputs(x, qkv_w, proj_w, proj_b, skip_x=False):
    """Host-side prep: bf16 casts + per-core weight shards (slab layout)."""
    import ml_dtypes
    bf16 = ml_dtypes.bfloat16

    if skip_x:
        x_bf = x                                             # already bf16
    else:
        x_bf = np.asarray(x, dtype=np.float32).astype(bf16)  # [8, 1024, 768]

    qkv_w = np.asarray(qkv_w, dtype=np.float32)
    proj_w = np.asarray(proj_w, dtype=np.float32)
    # cast to bf16 first (single pass per source region), then assemble
    # w_cat rows [q0,k0,..,q2,k2 | v (768) | q3,k3,..,q5,k5 | proj]
    qk_i = qkv_w[:2 * C].reshape(2, CB, P, C).transpose(1, 0, 2, 3).astype(
        bf16, order='C').reshape(2 * C, C)
    w_cat = np.concatenate(
        [qk_i[:C], qkv_w[2 * C:].astype(bf16), qk_i[C:],
         proj_w.astype(bf16)], axis=0)                        # [3072, 768]
    # core c's shard: chunk c of each slab, stacked (uneven slabs)
    parts = []
    row = 0
    for k in range(NSLAB):
        rows = SLAB_BLOCKS[k] * P
        parts.append(w_cat[row:row + rows].reshape(B, CHUNKS[k], C))
        row += rows
    shards = np.concatenate(parts, axis=1)

    proj_b = np.ascontiguousarray(np.asarray(proj_b, dtype=np.float32))
    return x_bf, shards, proj_b


def kernel(x, qkv_w, proj_w, proj_b):
    """Full inputs -> full output. x: [8, 1024, 768]."""
    global _NC_CACHE, _FAST_CACHE

    if _NC_CACHE is None:
        _NC_CACHE = build_attention_bass()
    nc = _NC_CACHE

    try:
        from concourse._compat import axon_active
        use_fast = axon_active()
    except Exception:
        use_fast = False

    if use_fast:
        try:
            import jax
            if _FAST_CACHE is None:
                _FAST_CACHE = _build_fast_runner(nc)
            in_names, in_sharding, run = _FAST_CACHE
            # Pipeline host prep with the (async) device transfers: x is
            # cast and shipped while the weight shards are still being
            # built. Per-core slices are contiguous, so the "concat" along
            # axis 0 is a free reshape -- no host copy.
            import ml_dtypes
            x_bf = np.asarray(x, dtype=np.float32).astype(ml_dtypes.bfloat16)
            devs = {"x": jax.device_put(x_bf.reshape(B * N, C), in_sharding)}
            _, shards, pb = _shard_inputs(x_bf, qkv_w, proj_w, proj_b,
                                          skip_x=True)
            devs["w_shard"] = jax.device_put(
                shards.reshape(B * SHARD_ROWS, C), in_sharding)
            devs["proj_b"] = jax.device_put(
                np.broadcast_to(pb, (B, C)).reshape(B * C), in_sharding)
            out = run([devs[nm] for nm in in_names])[0]
            return out.astype(np.float32)
        except Exception:
            _FAST_CACHE = None  # fall through to the portable path

    x_bf, shards, proj_b = _shard_inputs(x, qkv_w, proj_w, proj_b)

    from concourse.bass_utils import run_bass_kernel_spmd
    in_maps = [
        {"x": x_bf[b], "w_shard": shards[b], "proj_b": proj_b}
        for b in range(B)
    ]
    res = run_bass_kernel_spmd(nc, in_maps, core_ids=list(range(B)))
    outf = np.empty((B, N, C), dtype=np.float32)
    for b in range(B):
        outf[b] = res.results[b]["out"]
    return outf
